# revision 1
# baseline (speedup 1.0000x reference)
"""Trainium2 Bass kernel for nn_Linear_27608049779368.

Reference computation:
    out[b,c] = bias[c] + sum_o prod(x[:, idx_o], axis=2) @ W_o
    x [4096, 32], orders 1..3 with 32/496/4960 combos, C=128 classes.

Device algorithm (per core, data-parallel over batch, 8 cores x 512 rows):
    out.T = Wp.T @ exp(Inc.T @ log(x.T + c))        (all fp32)

  * c > -min(x) shifts features positive so products become sums of logs.
  * Inc [32, NK]: multiplicity of feature f in row-multiset T.  A single
    K=32 matmul per 128-row tile computes all the gathers AND products.
  * exp on ScalarE evacuates PSUM -> SBUF (only full-tensor elementwise
    pass; every other step is a matmul).
  * Wp [NK, 128] is host-transformed: expanding prod(x_f) =
    prod((x_f+c) - c) folds every cross term exactly into the weight row
    of the corresponding sub-multiset (all of which are themselves rows).
    The empty multiset is a constant row absorbing bias and c^o terms.
  * "Anti-mean" constant rows every 32 rows keep PSUM partial sums
    centered (numerics only; exactly compensated by a final restore row).

The result is mathematically exact in real arithmetic.  Measured on
hardware: absmax error 8.4e-3 on an output absmax of 15.9 (5.3e-4 of
scale), dominated by the PE fp32 matmul's internal per-product rounding
on the shift-inflated exp values; CoreSim cost model ~78us/core.
"""

import os
import sys
from itertools import combinations as _combinations

import numpy as np

for _p in ("/opt/trn_rl_repo", "/root/.axon_site/_ro/trn_rl_repo"):
    if os.path.isdir(_p) and _p not in sys.path:
        sys.path.insert(0, _p)
        break

import concourse.bass as bass
import concourse.bacc as bacc
import concourse.tile as tile
from concourse import mybir
from concourse.bass_utils import run_bass_kernel_spmd

N_CORES = 8
P = 128                 # partitions / tile size
EXP_FUSE = 3            # k-tiles per fused exp op (3 PSUM banks)
ANTI_MEAN_SPACING = 39  # centering const-row every N rows (39 -> 44 k-tiles)
F32 = mybir.dt.float32
F32R = mybir.dt.float32r
# fp32 matmuls stream at 4 cycles/row; float32r at 1 (N>=256).  The
# incidence matmul is made exact at fp32r speed by splitting log(x') into
# an 11-bit-mantissa high part plus residual (both fp32r-representable)
# and accumulating two fp32r matmuls in PSUM.
INC_FP32R_SPLIT = True


# ----------------------------------------------------------------------------
# Host-side math: rows, incidence, transformed weights
# ----------------------------------------------------------------------------

def _build_rows(idx_list, W_list, bias, c, F=32):
    """Build the row table (multisets), incidence and transformed weights.

    Returns Inc [F, NK] f32, Wp [NK, C] f64, rows (list of tuples).
    """
    C = W_list[0].shape[1]
    row_of = {}
    rows = []

    def get_row(t):
        r = row_of.get(t)
        if r is None:
            r = len(rows)
            row_of[t] = r
            rows.append(t)
        return r

    # Register original combos first, in given order, so the main mass of
    # each order sits in contiguous row blocks.
    combo_rows = []
    for idx, W in zip(idx_list, W_list):
        for k in range(idx.shape[0]):
            M = tuple(sorted(int(v) for v in idx[k]))
            combo_rows.append(get_row(M))

    Wp_contrib = []  # (row, coeff, W_vector)
    ci = 0
    const_acc = np.array(bias, np.float64).reshape(-1).copy()
    for idx, W in zip(idx_list, W_list):
        o = idx.shape[1]
        for k in range(idx.shape[0]):
            M = tuple(sorted(int(v) for v in idx[k]))
            Wk = W[k].astype(np.float64)
            for r in range(o, -1, -1):
                for sub in set(_combinations(M, r)):
                    cnt = sum(
                        1
                        for ss in _combinations(range(o), r)
                        if tuple(sorted(M[i] for i in ss)) == sub
                    )
                    coeff = ((-float(c)) ** (o - r)) * cnt
                    if r == 0:
                        const_acc += coeff * Wk
                    else:
                        Wp_contrib.append((get_row(sub), coeff, Wk))
            ci += 1

    const_row = get_row(())
    NK = len(rows)
    Inc = np.zeros((F, NK), np.float32)
    for r, t in enumerate(rows):
        for f in t:
            Inc[f, r] += 1.0
    Wp = np.zeros((NK, C), np.float64)
    for r, coeff, Wk in Wp_contrib:
        Wp[r] += coeff * Wk
    Wp[const_row] += const_acc
    return Inc, Wp, rows


def _add_anti_mean_rows(x, Inc, Wp, c, spacing):
    """Insert const rows every `spacing` rows cancelling the batch-mean mass
    of the preceding block; a final const row restores the total (exact)."""
    f32 = np.float32
    xp = np.maximum(x.astype(np.float64) + float(c), 1.0 / 64)
    Pv = np.exp(np.log(xp) @ Inc.astype(np.float64))     # [B, NK]
    mu = Pv.mean(axis=0)                                  # [NK]
    NK, C = Wp.shape
    F = Inc.shape[0]
    inc_cols, wp_rows = [], []
    total = np.zeros(C, np.float64)
    for t0 in range(0, NK, spacing):
        t1 = min(t0 + spacing, NK)
        inc_cols.append(Inc[:, t0:t1])
        wp_rows.append(Wp[t0:t1])
        mass = (mu[t0:t1, None] * Wp[t0:t1]).sum(axis=0)
        total += mass
        inc_cols.append(np.zeros((F, 1), f32))
        wp_rows.append((-mass)[None, :])
    inc_cols.append(np.zeros((F, 1), f32))
    wp_rows.append(total[None, :])
    return np.concatenate(inc_cols, axis=1), np.concatenate(wp_rows, axis=0)


def _split_big_weight_rows(Inc, Wp, thresh=32.0):
    """The PE's fp32 matmul multiplies with ~17-bit effective mantissas, so a
    product |P*W| is rounded at ~2^-17 relative.  Rows with large |W| (the
    constant / anti-mean / restore rows, whose P is exactly 1.0) dominate that
    error.  Split each such row into an 11-bit-mantissa hi part plus residual
    (both exactly representable through the truncated multiply) with a
    duplicated incidence column — mathematically identical, numerically clean.
    """
    mags = np.abs(Wp).max(axis=1)
    big = np.nonzero(mags > thresh)[0]
    if len(big) == 0:
        return Inc, Wp
    W32 = Wp.astype(np.float32)
    bits = W32.view(np.uint32)
    hi = (bits & np.uint32(0xFFFFF000)).view(np.float32)
    inc_cols = [Inc]
    wp_rows = [Wp.copy()]
    for r in big:
        lo = (W32[r].astype(np.float64) - hi[r].astype(np.float64))
        wp_rows[0][r] = hi[r]
        inc_cols.append(Inc[:, r:r + 1])
        wp_rows.append(lo[None, :])
    return np.concatenate(inc_cols, axis=1), np.concatenate(wp_rows, axis=0)


def _prepare(x, bias, W1, W2, W3, idx1, idx2, idx3):
    c = max(1.0, 0.5 - float(x.min()))
    Inc, Wp, _rows = _build_rows(
        [np.asarray(idx1), np.asarray(idx2), np.asarray(idx3)],
        [np.asarray(W1), np.asarray(W2), np.asarray(W3)],
        np.asarray(bias), c, F=np.asarray(x).shape[1])
    Inc, Wp = _add_anti_mean_rows(np.asarray(x), Inc, Wp, c, ANTI_MEAN_SPACING)
    NK = Inc.shape[1]
    nt = -(-NK // P)
    pad = nt * P - NK
    if pad:
        # dead rows: Inc col 0 -> exp(0)=1, Wp row 0 -> no contribution
        Inc = np.concatenate([Inc, np.zeros((Inc.shape[0], pad), np.float32)], axis=1)
        Wp = np.concatenate([Wp, np.zeros((pad, Wp.shape[1]), np.float64)], axis=0)
    return c, np.ascontiguousarray(Inc, np.float32), \
        np.ascontiguousarray(Wp.astype(np.float32)), nt


# ----------------------------------------------------------------------------
# Device kernel
# ----------------------------------------------------------------------------

def _build_nc(F, C, b_shard, nt, repeat=1):
    # Bacc (not plain Bass): finalize() runs the legalization passes —
    # notably generate_event_semaphores, which splits multi-sem waits
    # (TRN2 allows at most one sync wait per instruction).
    nc = bacc.Bacc(None, target_bir_lowering=False)
    d_xT = nc.declare_dram_parameter("xT", [F, b_shard], F32, isOutput=False)
    d_cv = nc.declare_dram_parameter("cvec", [F, 1], F32, isOutput=False)
    d_inc = nc.declare_dram_parameter("inc", [F, nt * P], F32, isOutput=False)
    d_wp = nc.declare_dram_parameter("wp", [nt * P, C], F32, isOutput=False)
    d_outT = nc.declare_dram_parameter("outT", [C, b_shard], F32, isOutput=True)

    with tile.TileContext(nc) as tc:
        with (
            tc.tile_pool(name="consts", bufs=1) as consts,
            tc.tile_pool(name="prods", bufs=1) as prods_pool,
            tc.tile_pool(name="wp_pool", bufs=8) as wp_pool,
            tc.tile_pool(name="psum_L", bufs=2, space="PSUM") as psum_L,
            tc.tile_pool(name="psum_out", bufs=1, space="PSUM") as psum_out,
        ):
            x_sb = consts.tile([F, b_shard], F32)
            nc.gpsimd.dma_start(out=x_sb, in_=d_xT[:, :])
            c_sb = consts.tile([F, 1], F32)
            nc.gpsimd.dma_start(out=c_sb, in_=d_cv[:, :])
            inc_sb = consts.tile([F, nt * P], F32)
            nc.gpsimd.dma_start(out=inc_sb, in_=d_inc[:, :])

            for _rep in range(repeat):
                _body_once(nc, tc, consts, prods_pool, wp_pool, psum_L,
                           psum_out, d_wp, d_outT, x_sb, c_sb, inc_sb,
                           F, C, b_shard, nt)
    nc.finalize()
    return nc


def _body_once(nc, tc, consts, prods_pool, wp_pool, psum_L, psum_out,
               d_wp, d_outT, x_sb, c_sb, inc_sb, F, C, b_shard, nt):
    # x' = max(x + c, 1/64); lx = log(x')
    xp_sb = consts.tile([F, b_shard], F32)
    nc.vector.tensor_scalar(
        out=xp_sb, in0=x_sb, scalar1=c_sb, scalar2=1.0 / 64,
        op0=mybir.AluOpType.add, op1=mybir.AluOpType.max)
    lx0 = consts.tile([F, b_shard], F32)
    nc.scalar.activation(lx0, xp_sb, mybir.ActivationFunctionType.Ln)
    # One Newton step refines the Ln table approximation to ~fp32 exactness:
    # l' = l + (x' * exp(-l) - 1).  The raw spline error (~1e-5) otherwise
    # dominates the end-to-end error (measured on hardware).
    e_neg = consts.tile([F, b_shard], F32)
    nc.scalar.activation(e_neg, lx0, mybir.ActivationFunctionType.Exp,
                         scale=-1.0)
    corr = consts.tile([F, b_shard], F32)
    nc.vector.tensor_mul(out=corr, in0=xp_sb, in1=e_neg)
    lx_sb = consts.tile([F, b_shard], F32)
    nc.vector.scalar_tensor_tensor(
        out=lx_sb, in0=corr, scalar=1.0, in1=lx0,
        op0=mybir.AluOpType.subtract, op1=mybir.AluOpType.add)

    if INC_FP32R_SPLIT:
        # lx = lx_hi + lx_res with both parts exactly fp32r
        # representable (the residual of a 12-bit round has at most
        # 12 significant bits), so two fp32r matmuls accumulating in
        # fp32 PSUM reproduce the fp32 matmul exactly.
        lx_hi = consts.tile([F, b_shard], F32R)
        nc.vector.tensor_copy(out=lx_hi, in_=lx_sb)
        lx_res = consts.tile([F, b_shard], F32)
        nc.vector.tensor_sub(out=lx_res, in0=lx_sb, in1=lx_hi)
        lx_res_r = consts.tile([F, b_shard], F32R)
        nc.vector.tensor_copy(out=lx_res_r, in_=lx_res)
        inc_r = consts.tile([F, nt * P], F32R)
        inc_mm = inc_r
        rhs_parts = [lx_hi, lx_res_r]
    else:
        inc_mm = inc_sb
        rhs_parts = [lx_sb]

    # log-sum matmuls + fused exp
    prods_tiles = []
    t = 0
    gi = 0
    while t < nt:
        g = min(EXP_FUSE, nt - t)
        if INC_FP32R_SPLIT:
            nc.vector.tensor_copy(out=inc_r[:, t * P:(t + g) * P],
                                  in_=inc_sb[:, t * P:(t + g) * P])
        L_ps = psum_L.tile([P, EXP_FUSE * b_shard], F32, tag="L")
        for j in range(g):
            for pi, rhs in enumerate(rhs_parts):
                nc.tensor.matmul(
                    L_ps[:, j * b_shard:(j + 1) * b_shard],
                    inc_mm[:, (t + j) * P:(t + j + 1) * P],
                    rhs,
                    start=(pi == 0), stop=(pi == len(rhs_parts) - 1))
        pg = prods_pool.tile([P, g * b_shard], F32, tag=f"pg{gi}")
        nc.scalar.activation(
            pg, L_ps[:, :g * b_shard], mybir.ActivationFunctionType.Exp)
        for j in range(g):
            prods_tiles.append(pg[:, j * b_shard:(j + 1) * b_shard])
        t += g
        gi += 1

    # main contraction: outT += Wp_tile.T @ prods_tile
    out_ps = psum_out.tile([C, b_shard], F32)
    for t2 in range(nt):
        wp_t = wp_pool.tile([P, C], F32, tag="wp")
        nc.gpsimd.dma_start(out=wp_t, in_=d_wp[t2 * P:(t2 + 1) * P, :])
        nc.tensor.matmul(
            out_ps, wp_t, prods_tiles[t2],
            start=(t2 == 0), stop=(t2 == nt - 1))

    out_sb = consts.tile([C, b_shard], F32)
    nc.vector.tensor_copy(out=out_sb, in_=out_ps)
    nc.gpsimd.dma_start(out=d_outT[:, :], in_=out_sb)


_nc_cache = {}


def _get_nc(F, C, b_shard, nt, repeat=1):
    key = (F, C, b_shard, nt, repeat)
    if key not in _nc_cache:
        _nc_cache[key] = _build_nc(F, C, b_shard, nt, repeat)
    return _nc_cache[key]


def _make_in_maps(x, c, Inc, Wp, b_shard):
    F = x.shape[1]
    cvec = np.full((F, 1), c, np.float32)
    in_maps = []
    for i in range(N_CORES):
        sh = np.ascontiguousarray(
            x[i * b_shard:(i + 1) * b_shard].T.astype(np.float32))
        in_maps.append({"xT": sh, "cvec": cvec, "inc": Inc, "wp": Wp})
    return in_maps


def kernel(x, bias, W1, W2, W3, idx1, idx2, idx3, _trace=False):
    x = np.asarray(x, np.float32)
    B, F = x.shape
    C = np.asarray(W1).shape[1]
    assert B % N_CORES == 0
    b_shard = B // N_CORES

    c, Inc, Wp, nt = _prepare(x, bias, W1, W2, W3, idx1, idx2, idx3)
    nc = _get_nc(F, C, b_shard, nt)
    in_maps = _make_in_maps(x, c, Inc, Wp, b_shard)
    res = run_bass_kernel_spmd(nc, in_maps, list(range(N_CORES)), trace=_trace)
    out = np.empty((B, C), np.float32)
    for i in range(N_CORES):
        out[i * b_shard:(i + 1) * b_shard] = res.results[i]["outT"].T
    if _trace:
        kernel.last_results = res
    return out



# revision 2
# speedup vs baseline: 1.1482x; 1.1482x over previous
"""Trainium2 Bass kernel for nn_Linear_27608049779368 (lean body).

Reference computation:
    out[b,c] = bias[c] + sum_o prod(x[:, idx_o], axis=2) @ W_o
    x [4096, 32], orders 1..3 with 32/496/4960 combos, C=128 classes.

Device algorithm (per core, data-parallel over batch, 8 cores x 512 rows):
    out.T = Wp.T @ exp(Inc.T @ log(x.T + c))        (fp32 / fp32r)

v2 vs baseline: minimal instruction & DMA count per body.
  * 4 input DMAs total; Wp shipped pre-transposed so it is one contiguous
    [128, nt*128] HWDGE transfer (128 x 22 KB lines).
  * Prep is ONE activation: lx = Ln(x + c) (bias folded into the ACT op,
    the max(.,1/64) clamp never binds since min(x+c) = 0.5), written
    straight to an fp32r tile (rounds to 11 mantissa bits).
  * Inc matmul: single fp32r pass.  Numpy model of 11-bit lx rounding +
    fp32-PE main matmul: 1.5e-3 rel error (budget 2e-2).
  * No anti-mean / big-row-split machinery (fp32 PSUM accumulation
    without it models at ~1.2e-3 rel).
  * exp fused 4 tiles per ACTIVATE ([128, 2048] PSUM->SBUF).
  * Main contraction plain fp32 (17-bit effective products).
  * PSUM->SBUF evacuation on ScalarE (Copy), not DVE.
"""

import os
import sys

import numpy as np

for _p in ("/opt/trn_rl_repo", "/root/.axon_site/_ro/trn_rl_repo"):
    if os.path.isdir(_p) and _p not in sys.path:
        sys.path.insert(0, _p)
        break

import concourse.bass as bass
import concourse.bacc as bacc
import concourse.tile as tile
from concourse import mybir
from concourse.bass_utils import run_bass_kernel_spmd

N_CORES = 8
P = 128
EXP_FUSE = 7            # k-tiles per fused exp op (7 PSUM banks + 1 out bank)
NEWTON = False          # one Newton step on Ln (3 extra ops) if needed
F32 = mybir.dt.float32
F32R = mybir.dt.float32r


# ----------------------------------------------------------------------------
# Host-side math: rows, incidence, transformed weights
# ----------------------------------------------------------------------------

def _build_rows(idx_list, W_list, bias, c, F=32):
    """Row table (multisets), incidence Inc [F, NK] and transformed weights
    Wp [NK, C] (f64) such that  out = Wp.T @ exp(Inc.T @ log(x + c))."""
    from itertools import combinations as _comb
    C = W_list[0].shape[1]
    row_of = {}
    rows = []

    def get_row(t):
        r = row_of.get(t)
        if r is None:
            r = len(rows)
            row_of[t] = r
            rows.append(t)
        return r

    for idx, W in zip(idx_list, W_list):
        for k in range(idx.shape[0]):
            get_row(tuple(sorted(int(v) for v in idx[k])))

    Wp_contrib = []
    const_acc = np.array(bias, np.float64).reshape(-1).copy()
    for idx, W in zip(idx_list, W_list):
        o = idx.shape[1]
        for k in range(idx.shape[0]):
            M = tuple(sorted(int(v) for v in idx[k]))
            Wk = W[k].astype(np.float64)
            for r in range(o, -1, -1):
                for sub in set(_comb(M, r)):
                    cnt = sum(
                        1
                        for ss in _comb(range(o), r)
                        if tuple(sorted(M[i] for i in ss)) == sub
                    )
                    coeff = ((-float(c)) ** (o - r)) * cnt
                    if r == 0:
                        const_acc += coeff * Wk
                    else:
                        Wp_contrib.append((get_row(sub), coeff, Wk))

    const_row = get_row(())
    NK = len(rows)
    Inc = np.zeros((F, NK), np.float32)
    for r, t in enumerate(rows):
        for f in t:
            Inc[f, r] += 1.0
    Wp = np.zeros((NK, C), np.float64)
    for r, coeff, Wk in Wp_contrib:
        Wp[r] += coeff * Wk
    Wp[const_row] += const_acc
    return Inc, Wp


def _prepare(x, bias, W1, W2, W3, idx1, idx2, idx3):
    x = np.asarray(x)
    F = x.shape[1]
    C = np.asarray(W1).shape[1]
    c = max(1.0, 0.5 - float(x.min()))
    Inc, Wp = _build_rows(
        [np.asarray(idx1), np.asarray(idx2), np.asarray(idx3)],
        [np.asarray(W1), np.asarray(W2), np.asarray(W3)],
        np.asarray(bias), c, F=F)
    NK = Inc.shape[1]
    nt = -(-NK // P)
    pad = nt * P - NK
    if pad:
        Inc = np.concatenate([Inc, np.zeros((F, pad), np.float32)], axis=1)
        Wp = np.concatenate([Wp, np.zeros((pad, Wp.shape[1]), np.float64)], axis=0)
    # Wp packed so the whole thing is one [128, nt*128] contiguous DMA:
    # partition p, block t, col j  =  Wp[t*128 + p, j].
    WpA = np.ascontiguousarray(
        Wp.astype(np.float32).reshape(nt, P, C).transpose(1, 0, 2).reshape(P, nt * C))
    return c, np.ascontiguousarray(Inc), WpA, nt


# ----------------------------------------------------------------------------
# Device kernel
# ----------------------------------------------------------------------------

def _body_once(nc, tc, consts, prods_pool, psum_L, psum_out,
               d_outT, x_sb, inc_sb, wp_sb, F, C, b_shard, nt):
    # x_sb already holds x + c (host-shifted); min = 0.5 so no clamp.
    lx = consts.tile([F, b_shard], F32R)
    if not NEWTON:
        nc.scalar.activation(lx, x_sb, mybir.ActivationFunctionType.Ln)
    else:
        lx0 = consts.tile([F, b_shard], F32)
        nc.scalar.activation(lx0, x_sb, mybir.ActivationFunctionType.Ln)
        e_neg = consts.tile([F, b_shard], F32)
        nc.scalar.activation(e_neg, lx0, mybir.ActivationFunctionType.Exp,
                             scale=-1.0)
        corr = consts.tile([F, b_shard], F32)
        nc.vector.tensor_mul(out=corr, in0=x_sb, in1=e_neg)
        nc.vector.scalar_tensor_tensor(
            out=lx, in0=corr, scalar=1.0, in1=lx0,
            op0=mybir.AluOpType.subtract, op1=mybir.AluOpType.add)

    out_ps = psum_out.tile([C, b_shard], F32)
    G = -(-nt // EXP_FUSE)
    for g in range(G):
        w = min(EXP_FUSE, nt - g * EXP_FUSE)
        L_ps = psum_L.tile([P, EXP_FUSE * b_shard], F32, tag="L")
        for k in range(w):
            t = g * EXP_FUSE + k
            nc.tensor.matmul(
                L_ps[:, k * b_shard:(k + 1) * b_shard],
                inc_sb[:, t * P:(t + 1) * P],
                lx,
                start=True, stop=True)
        pg = prods_pool.tile([P, EXP_FUSE * b_shard], F32, tag="pg")
        nc.scalar.activation(
            pg[:, :w * b_shard], L_ps[:, :w * b_shard],
            mybir.ActivationFunctionType.Exp)
        for k in range(w):
            t = g * EXP_FUSE + k
            nc.tensor.matmul(
                out_ps, wp_sb[:, t * C:(t + 1) * C],
                pg[:, k * b_shard:(k + 1) * b_shard],
                start=(t == 0), stop=(t == nt - 1))

    out_sb = consts.tile([C, b_shard], F32)
    nc.scalar.activation(out_sb, out_ps, mybir.ActivationFunctionType.Copy)
    nc.sync.dma_start(out=d_outT[:, :], in_=out_sb)


_nc_cache = {}


def _get_nc(F, C, b_shard, nt, c, repeat=1):
    key = (F, C, b_shard, nt, float(c), repeat)
    if key not in _nc_cache:
        _nc_cache[key] = _build_nc_with_c(F, C, b_shard, nt, c, repeat)
    return _nc_cache[key]


def _build_nc_with_c(F, C, b_shard, nt, c, repeat):
    nc = bacc.Bacc(None, target_bir_lowering=False)
    d_xT = nc.declare_dram_parameter("xT", [F, b_shard], F32, isOutput=False)
    d_inc = nc.declare_dram_parameter("inc", [F, nt * P], F32R, isOutput=False)
    d_wp = nc.declare_dram_parameter("wp", [P, nt * C], F32, isOutput=False)
    d_outT = nc.declare_dram_parameter("outT", [C, b_shard], F32, isOutput=True)

    with tile.TileContext(nc) as tc:
        with (
            tc.tile_pool(name="consts", bufs=1) as consts,
            tc.tile_pool(name="prods", bufs=3) as prods_pool,
            tc.tile_pool(name="psum_L", bufs=1, space="PSUM") as psum_L,
            tc.tile_pool(name="psum_out", bufs=1, space="PSUM") as psum_out,
        ):
            x_sb = consts.tile([F, b_shard], F32)
            nc.sync.dma_start(out=x_sb, in_=d_xT[:, :])
            inc_sb = consts.tile([F, nt * P], F32R)
            nc.sync.dma_start(out=inc_sb, in_=d_inc[:, :])
            wp_sb = consts.tile([P, nt * C], F32)
            nc.sync.dma_start(out=wp_sb, in_=d_wp[:, :])

            for _rep in range(repeat):
                _body_once(nc, tc, consts, prods_pool, psum_L, psum_out,
                           d_outT, x_sb, inc_sb, wp_sb, F, C, b_shard, nt)
    nc.finalize()
    _merge_act_table_loads(nc)
    _strip_overhead(nc)
    return nc


def _strip_overhead(nc):
    """Drop setup/tail instructions that don't affect this kernel's result:
    the unused const-AP memsets (only the f32 0.0 bias constant is read)
    and the end-block all-engine barrier cascade (drains / event semaphores
    that reference only barrier_* rendezvous semaphores).  The data-bearing
    completion waits (DMA / engine sems) are kept, so the output DMA is
    still guaranteed complete at NEFF end.  Validated under CoreSim's race
    detector."""
    keep_memset = {"const-float32-0.0"}
    blocks = list(nc.m.functions[0].blocks)
    for bi, b in enumerate(blocks):
        last = bi == len(blocks) - 1
        keep = []
        for inst in b.instructions:
            nm = type(inst).__name__
            si = inst.sync_info
            sems = []
            if si is not None:
                sems += [w.ant_name for w in si.on_wait]
                sems += [u.ant_name for u in si.on_update]
            only_barrier = bool(sems) and all(
                s.startswith("barrier_") for s in sems)
            if nm == "InstMemset" and str(
                    getattr(inst.outs[0], "memref", "")) not in keep_memset:
                continue
            if last and nm in ("InstDrain", "InstEventSemaphore") and (
                    only_barrier or (nm == "InstDrain" and not sems)):
                continue
            keep.append(inst)
        b.instructions[:] = keep


def _merge_act_table_loads(nc):
    """All activations here (Ln, Exp, Copy) live in one table set
    (natural_log_exp_and_others); keep a single load of that set instead
    of the per-function flip-flop bacc emits."""
    from concourse.hw_specs import get_activation_tables
    T = mybir.ActivationFunctionType
    tabs = get_activation_tables(nc.m.arch)
    combined = None
    for i, fns in enumerate(tabs.values()):
        if {T.Ln, T.Exp, T.Copy} <= fns:
            combined = i
            break
    if combined is None:
        return
    first = True
    for b in nc.m.functions[0].blocks:
        keep = []
        for inst in b.instructions:
            if isinstance(inst, mybir.InstLoadActFuncSet):
                if first:
                    inst.act_func_set_id = combined
                    first = False
                    keep.append(inst)
                # later loads dropped: set already resident
            else:
                keep.append(inst)
        b.instructions[:] = keep


def _make_in_maps(x, c, Inc, WpA, b_shard):
    F = x.shape[1]
    in_maps = []
    for i in range(N_CORES):
        sh = np.ascontiguousarray(
            (x[i * b_shard:(i + 1) * b_shard].T + np.float32(c))
            .astype(np.float32))
        in_maps.append({"xT": sh, "inc": Inc, "wp": WpA})
    return in_maps


def kernel(x, bias, W1, W2, W3, idx1, idx2, idx3, _trace=False):
    x = np.asarray(x, np.float32)
    B, F = x.shape
    C = np.asarray(W1).shape[1]
    assert B % N_CORES == 0
    b_shard = B // N_CORES

    c, Inc, WpA, nt = _prepare(x, bias, W1, W2, W3, idx1, idx2, idx3)
    nc = _get_nc(F, C, b_shard, nt, c)
    in_maps = _make_in_maps(x, c, Inc, WpA, b_shard)
    res = run_bass_kernel_spmd(nc, in_maps, list(range(N_CORES)), trace=_trace)
    out = np.empty((B, C), np.float32)
    for i in range(N_CORES):
        out[i * b_shard:(i + 1) * b_shard] = res.results[i]["outT"].T
    if _trace:
        kernel.last_results = res
    return out


# revision 6
# speedup vs baseline: 1.2274x; 1.0690x over previous
"""Trainium2 Bass kernel for nn_Linear_27608049779368 (lean body).

Reference computation:
    out[b,c] = bias[c] + sum_o prod(x[:, idx_o], axis=2) @ W_o
    x [4096, 32], orders 1..3 with 32/496/4960 combos, C=128 classes.

Device algorithm (per core, data-parallel over batch, 8 cores x 512 rows):
    out.T = Wp.T @ exp(Inc.T @ log(x.T + c))        (fp32 / fp32r)

v2 vs baseline: minimal instruction & DMA count per body.
  * 4 input DMAs total; Wp shipped pre-transposed so it is one contiguous
    [128, nt*128] HWDGE transfer (128 x 22 KB lines).
  * Prep is ONE activation: lx = Ln(x + c) (bias folded into the ACT op,
    the max(.,1/64) clamp never binds since min(x+c) = 0.5), written
    straight to an fp32r tile (rounds to 11 mantissa bits).
  * Inc matmul: single fp32r pass.  Numpy model of 11-bit lx rounding +
    fp32-PE main matmul: 1.5e-3 rel error (budget 2e-2).
  * No anti-mean / big-row-split machinery (fp32 PSUM accumulation
    without it models at ~1.2e-3 rel).
  * exp fused 4 tiles per ACTIVATE ([128, 2048] PSUM->SBUF).
  * Main contraction plain fp32 (17-bit effective products).
  * PSUM->SBUF evacuation on ScalarE (Copy), not DVE.
"""

import os
import sys

import numpy as np

for _p in ("/opt/trn_rl_repo", "/root/.axon_site/_ro/trn_rl_repo"):
    if os.path.isdir(_p) and _p not in sys.path:
        sys.path.insert(0, _p)
        break

import concourse.bass as bass
import concourse.bacc as bacc
import concourse.tile as tile
from concourse import mybir
from concourse.bass_utils import run_bass_kernel_spmd

N_CORES = 8
P = 128
EXP_FUSE = 7            # k-tiles per fused exp op (7 PSUM banks + 1 out bank)
NEWTON = False          # one Newton step on Ln (3 extra ops) if needed
F32 = mybir.dt.float32
F32R = mybir.dt.float32r


# ----------------------------------------------------------------------------
# Host-side math: rows, incidence, transformed weights
# ----------------------------------------------------------------------------

def _build_rows(idx_list, W_list, bias, c, F=32):
    """Row table (multisets), incidence Inc [F, NK] and transformed weights
    Wp [NK, C] (f64) such that  out = Wp.T @ exp(Inc.T @ log(x + c))."""
    from itertools import combinations as _comb
    C = W_list[0].shape[1]
    row_of = {}
    rows = []

    def get_row(t):
        r = row_of.get(t)
        if r is None:
            r = len(rows)
            row_of[t] = r
            rows.append(t)
        return r

    for idx, W in zip(idx_list, W_list):
        for k in range(idx.shape[0]):
            get_row(tuple(sorted(int(v) for v in idx[k])))

    Wp_contrib = []
    const_acc = np.array(bias, np.float64).reshape(-1).copy()
    for idx, W in zip(idx_list, W_list):
        o = idx.shape[1]
        for k in range(idx.shape[0]):
            M = tuple(sorted(int(v) for v in idx[k]))
            Wk = W[k].astype(np.float64)
            for r in range(o, -1, -1):
                for sub in set(_comb(M, r)):
                    cnt = sum(
                        1
                        for ss in _comb(range(o), r)
                        if tuple(sorted(M[i] for i in ss)) == sub
                    )
                    coeff = ((-float(c)) ** (o - r)) * cnt
                    if r == 0:
                        const_acc += coeff * Wk
                    else:
                        Wp_contrib.append((get_row(sub), coeff, Wk))

    const_row = get_row(())
    NK = len(rows)
    Inc = np.zeros((F, NK), np.float32)
    for r, t in enumerate(rows):
        for f in t:
            Inc[f, r] += 1.0
    Wp = np.zeros((NK, C), np.float64)
    for r, coeff, Wk in Wp_contrib:
        Wp[r] += coeff * Wk
    Wp[const_row] += const_acc
    return Inc, Wp


def _prepare(x, bias, W1, W2, W3, idx1, idx2, idx3):
    x = np.asarray(x)
    F = x.shape[1]
    C = np.asarray(W1).shape[1]
    c = max(1.0, 0.5 - float(x.min()))
    Inc, Wp = _build_rows(
        [np.asarray(idx1), np.asarray(idx2), np.asarray(idx3)],
        [np.asarray(W1), np.asarray(W2), np.asarray(W3)],
        np.asarray(bias), c, F=F)
    NK = Inc.shape[1]
    nt = -(-NK // P)
    pad = nt * P - NK
    if pad:
        Inc = np.concatenate([Inc, np.zeros((F, pad), np.float32)], axis=1)
        Wp = np.concatenate([Wp, np.zeros((pad, Wp.shape[1]), np.float64)], axis=0)
    # Wp packed so the whole thing is one [128, nt*128+1] contiguous DMA:
    # partition p, block t, col j  =  Wp[t*128 + p, j].  The final column is
    # zero and serves as the Exp activation's bias AP (avoids the const-AP
    # memset + init barrier).
    WpA = np.ascontiguousarray(np.concatenate([
        Wp.astype(np.float32).reshape(nt, P, C).transpose(1, 0, 2).reshape(P, nt * C),
        np.zeros((P, 1), np.float32)], axis=1))
    return c, np.ascontiguousarray(Inc), WpA, nt


# ----------------------------------------------------------------------------
# Device kernel
# ----------------------------------------------------------------------------

def _body_once(nc, tc, consts, prods_pool, psum_L, psum_out,
               d_outT, x_sb, inc_sb, wp_sb, F, C, b_shard, nt):
    # x_sb already holds x + c (host-shifted); min = 0.5 so no clamp.
    lx = consts.tile([F, b_shard], F32R)
    zb_x = x_sb[:, b_shard:b_shard + 1]      # shipped zero column
    zb_w = wp_sb[:, nt * C:nt * C + 1]       # shipped zero column
    if not NEWTON:
        nc.scalar.activation(lx, x_sb[:, :b_shard],
                             mybir.ActivationFunctionType.Ln, bias=zb_x)
    else:
        lx0 = consts.tile([F, b_shard], F32)
        nc.scalar.activation(lx0, x_sb, mybir.ActivationFunctionType.Ln)
        e_neg = consts.tile([F, b_shard], F32)
        nc.scalar.activation(e_neg, lx0, mybir.ActivationFunctionType.Exp,
                             scale=-1.0)
        corr = consts.tile([F, b_shard], F32)
        nc.vector.tensor_mul(out=corr, in0=x_sb, in1=e_neg)
        nc.vector.scalar_tensor_tensor(
            out=lx, in0=corr, scalar=1.0, in1=lx0,
            op0=mybir.AluOpType.subtract, op1=mybir.AluOpType.add)

    out_ps = psum_out.tile([C, b_shard], F32)
    G = -(-nt // EXP_FUSE)
    for g in range(G):
        w = min(EXP_FUSE, nt - g * EXP_FUSE)
        L_ps = psum_L.tile([P, EXP_FUSE * b_shard], F32, tag="L")
        for k in range(w):
            t = g * EXP_FUSE + k
            nc.tensor.matmul(
                L_ps[:, k * b_shard:(k + 1) * b_shard],
                inc_sb[:, t * P:(t + 1) * P],
                lx,
                start=True, stop=True)
        pg = prods_pool.tile([P, EXP_FUSE * b_shard], F32, tag="pg")
        nc.scalar.activation(
            pg[:, :w * b_shard], L_ps[:, :w * b_shard],
            mybir.ActivationFunctionType.Exp, bias=zb_w)
        for k in range(w):
            t = g * EXP_FUSE + k
            nc.tensor.matmul(
                out_ps, wp_sb[:, t * C:(t + 1) * C],
                pg[:, k * b_shard:(k + 1) * b_shard],
                start=(t == 0), stop=(t == nt - 1))

    out_sb = consts.tile([C, b_shard], F32)
    nc.scalar.activation(out_sb, out_ps, mybir.ActivationFunctionType.Copy)
    nc.sync.dma_start(out=d_outT[:, :], in_=out_sb)


_nc_cache = {}


def _get_nc(F, C, b_shard, nt, c, repeat=1):
    key = (F, C, b_shard, nt, float(c), repeat)
    if key not in _nc_cache:
        _nc_cache[key] = _build_nc_with_c(F, C, b_shard, nt, c, repeat)
    return _nc_cache[key]


def _build_nc_with_c(F, C, b_shard, nt, c, repeat):
    nc = bacc.Bacc(None, target_bir_lowering=False)
    # inc carries [Inc | x'+c | zero-col] so input staging is one DMA fewer;
    # the x' slice is read back as plain f32 via bitcast (DMA is a byte copy).
    d_inc = nc.declare_dram_parameter(
        "inc", [F, nt * P + b_shard + 1], F32R, isOutput=False)
    d_wp = nc.declare_dram_parameter("wp", [P, nt * C + 1], F32, isOutput=False)
    d_outT = nc.declare_dram_parameter("outT", [C, b_shard], F32, isOutput=True)

    with tile.TileContext(nc) as tc:
        with (
            tc.tile_pool(name="consts", bufs=1) as consts,
            tc.tile_pool(name="prods", bufs=3) as prods_pool,
            tc.tile_pool(name="psum_L", bufs=1, space="PSUM") as psum_L,
            tc.tile_pool(name="psum_out", bufs=1, space="PSUM") as psum_out,
        ):
            inc_sb = consts.tile([F, nt * P + b_shard + 1], F32R)
            nc.sync.dma_start(out=inc_sb, in_=d_inc[:, :])
            x_sb = inc_sb[:, nt * P:nt * P + b_shard + 1].bitcast(F32)
            wp_sb = consts.tile([P, nt * C + 1], F32)
            nc.sync.dma_start(out=wp_sb, in_=d_wp[:, :])

            for _rep in range(repeat):
                _body_once(nc, tc, consts, prods_pool, psum_L, psum_out,
                           d_outT, x_sb, inc_sb, wp_sb, F, C, b_shard, nt)
    nc.finalize()
    _merge_act_table_loads(nc)
    _strip_overhead(nc)
    return nc


def _strip_overhead(nc):
    """Drop setup/tail instructions that don't affect this kernel's result:
    the unused const-AP memsets (only the f32 0.0 bias constant is read)
    and the end-block all-engine barrier cascade (drains / event semaphores
    that reference only barrier_* rendezvous semaphores).  The data-bearing
    completion waits (DMA / engine sems) are kept, so the output DMA is
    still guaranteed complete at NEFF end.  Validated under CoreSim's race
    detector."""
    blocks = list(nc.m.functions[0].blocks)
    for b in blocks:
        keep = []
        for inst in b.instructions:
            nm = type(inst).__name__
            si = inst.sync_info
            sems = []
            if si is not None:
                sems += [w.ant_name for w in si.on_wait]
                sems += [u.ant_name for u in si.on_update]
            only_barrier = bool(sems) and all(
                s.startswith("barrier_") for s in sems)
            if nm == "InstMemset" and "const-" in str(
                    getattr(inst.outs[0], "memref", "")):
                continue
            if nm in ("InstDrain", "InstEventSemaphore") and (
                    only_barrier or (nm == "InstDrain" and not sems)):
                continue
            # the remaining SP drain (wait ACT==all) is subsumed by the
            # out-DMA completion waits; InstISA is a Pool marker and no
            # Pool work remains after the memset removal
            if nm == "InstISA":
                continue
            if nm == "InstDrain":
                continue
            keep.append(inst)
        b.instructions[:] = keep
    # merge the (now purely sequential) blocks into one and drop the
    # inter-block branches; per-engine instruction order is preserved
    merged = []
    for b in blocks:
        for inst in b.instructions:
            if type(inst).__name__ == "InstUnconditionalBranch":
                continue
            merged.append(inst)
    blocks[0].instructions[:] = merged
    nc.m.functions[0].blocks[:] = [blocks[0]]


def _merge_act_table_loads(nc):
    """All activations here (Ln, Exp, Copy) live in one table set
    (natural_log_exp_and_others); keep a single load of that set instead
    of the per-function flip-flop bacc emits."""
    from concourse.hw_specs import get_activation_tables
    T = mybir.ActivationFunctionType
    tabs = get_activation_tables(nc.m.arch)
    combined = None
    for i, fns in enumerate(tabs.values()):
        if {T.Ln, T.Exp, T.Copy} <= fns:
            combined = i
            break
    if combined is None:
        return
    first = True
    for b in nc.m.functions[0].blocks:
        keep = []
        for inst in b.instructions:
            if isinstance(inst, mybir.InstLoadActFuncSet):
                if first:
                    inst.act_func_set_id = combined
                    first = False
                    keep.append(inst)
                # later loads dropped: set already resident
            else:
                keep.append(inst)
        b.instructions[:] = keep


def _make_in_maps(x, c, Inc, WpA, b_shard):
    F = x.shape[1]
    in_maps = []
    for i in range(N_CORES):
        sh = np.ascontiguousarray(np.concatenate([
            Inc,
            (x[i * b_shard:(i + 1) * b_shard].T + np.float32(c))
            .astype(np.float32),
            np.zeros((F, 1), np.float32)], axis=1))
        in_maps.append({"inc": sh, "wp": WpA})
    return in_maps


def kernel(x, bias, W1, W2, W3, idx1, idx2, idx3, _trace=False):
    x = np.asarray(x, np.float32)
    B, F = x.shape
    C = np.asarray(W1).shape[1]
    assert B % N_CORES == 0
    b_shard = B // N_CORES

    c, Inc, WpA, nt = _prepare(x, bias, W1, W2, W3, idx1, idx2, idx3)
    nc = _get_nc(F, C, b_shard, nt, c)
    in_maps = _make_in_maps(x, c, Inc, WpA, b_shard)
    res = run_bass_kernel_spmd(nc, in_maps, list(range(N_CORES)), trace=_trace)
    out = np.empty((B, C), np.float32)
    for i in range(N_CORES):
        out[i * b_shard:(i + 1) * b_shard] = res.results[i]["outT"].T
    if _trace:
        kernel.last_results = res
    return out


# revision 7
# speedup vs baseline: 1.4782x; 1.2043x over previous
"""Trainium2 Bass kernel for nn_Linear_27608049779368 (lean body).

Reference computation:
    out[b,c] = bias[c] + sum_o prod(x[:, idx_o], axis=2) @ W_o
    x [4096, 32], orders 1..3 with 32/496/4960 combos, C=128 classes.

Device algorithm (per core, data-parallel over batch, 8 cores x 512 rows):
    out.T = Wp.T @ exp(Inc.T @ log(x.T + c))        (fp32 / fp32r)

v2 vs baseline: minimal instruction & DMA count per body.
  * 4 input DMAs total; Wp shipped pre-transposed so it is one contiguous
    [128, nt*128] HWDGE transfer (128 x 22 KB lines).
  * Prep is ONE activation: lx = Ln(x + c) (bias folded into the ACT op,
    the max(.,1/64) clamp never binds since min(x+c) = 0.5), written
    straight to an fp32r tile (rounds to 11 mantissa bits).
  * Both matmul stages run fp32r (single pass each).  Per-feature shifts
    c_f = 0.5 - min_b(x[:,f]) keep the product magnitudes small enough
    that the 11-bit operand rounding lands at 1.34e-2 rel error on HW
    (budget 2e-2); a global shift would be 2.5e-2.
  * No anti-mean / big-row-split machinery (fp32 PSUM accumulation
    without it models at ~1.2e-3 rel).
  * exp fused 4 tiles per ACTIVATE ([128, 2048] PSUM->SBUF).
  * Main contraction plain fp32 (17-bit effective products).
  * PSUM->SBUF evacuation on ScalarE (Copy), not DVE.
"""

import os
import sys

import numpy as np

for _p in ("/opt/trn_rl_repo", "/root/.axon_site/_ro/trn_rl_repo"):
    if os.path.isdir(_p) and _p not in sys.path:
        sys.path.insert(0, _p)
        break

import concourse.bass as bass
import concourse.bacc as bacc
import concourse.tile as tile
from concourse import mybir
from concourse.bass_utils import run_bass_kernel_spmd

N_CORES = 8
P = 128
EXP_FUSE = 7            # k-tiles per fused exp op (7 PSUM banks + 1 out bank)
NEWTON = False          # one Newton step on Ln (3 extra ops) if needed
F32 = mybir.dt.float32
F32R = mybir.dt.float32r


# ----------------------------------------------------------------------------
# Host-side math: rows, incidence, transformed weights
# ----------------------------------------------------------------------------

def _build_rows(idx_list, W_list, bias, c, F=32):
    """Row table (multisets), incidence Inc [F, NK] and transformed weights
    Wp [NK, C] (f64) such that  out = Wp.T @ exp(Inc.T @ log(x + c))."""
    from itertools import combinations as _comb
    C = W_list[0].shape[1]
    row_of = {}
    rows = []

    def get_row(t):
        r = row_of.get(t)
        if r is None:
            r = len(rows)
            row_of[t] = r
            rows.append(t)
        return r

    for idx, W in zip(idx_list, W_list):
        for k in range(idx.shape[0]):
            get_row(tuple(sorted(int(v) for v in idx[k])))

    Wp_contrib = []
    const_acc = np.array(bias, np.float64).reshape(-1).copy()
    cf = np.asarray(c, np.float64).reshape(-1)
    for idx, W in zip(idx_list, W_list):
        o = idx.shape[1]
        for k in range(idx.shape[0]):
            M = tuple(sorted(int(v) for v in idx[k]))
            Wk = W[k].astype(np.float64)
            for r in range(o, -1, -1):
                for sub in _comb(M, r):
                    # all index sets are distinct combinations, so the
                    # expansion coefficient is just the product of the
                    # per-feature shifts of the removed features
                    sset = set(sub)
                    coeff = 1.0
                    for f in M:
                        if f not in sset:
                            coeff *= -cf[f]
                    if r == 0:
                        const_acc += coeff * Wk
                    else:
                        Wp_contrib.append((get_row(tuple(sub)), coeff, Wk))

    const_row = get_row(())
    NK = len(rows)
    Inc = np.zeros((F, NK), np.float32)
    for r, t in enumerate(rows):
        for f in t:
            Inc[f, r] += 1.0
    Wp = np.zeros((NK, C), np.float64)
    for r, coeff, Wk in Wp_contrib:
        Wp[r] += coeff * Wk
    Wp[const_row] += const_acc
    return Inc, Wp


def _prepare(x, bias, W1, W2, W3, idx1, idx2, idx3):
    x = np.asarray(x)
    F = x.shape[1]
    C = np.asarray(W1).shape[1]
    # per-feature shifts keep the exp-domain product magnitudes ~2.4x
    # smaller than a global shift, which is what lets the main contraction
    # run in fp32r (11-bit operands) within the 2e-2 error budget.
    c = np.maximum(1.0, 0.5 - x.min(axis=0).astype(np.float64))
    Inc, Wp = _build_rows(
        [np.asarray(idx1), np.asarray(idx2), np.asarray(idx3)],
        [np.asarray(W1), np.asarray(W2), np.asarray(W3)],
        np.asarray(bias), c, F=F)
    NK = Inc.shape[1]
    nt = -(-NK // P)
    pad = nt * P - NK
    if pad:
        Inc = np.concatenate([Inc, np.zeros((F, pad), np.float32)], axis=1)
        Wp = np.concatenate([Wp, np.zeros((pad, Wp.shape[1]), np.float64)], axis=0)
    # Wp packed so the whole thing is one [128, nt*128+1] contiguous DMA:
    # partition p, block t, col j  =  Wp[t*128 + p, j].  The final column is
    # zero and serves as the Exp activation's bias AP (avoids the const-AP
    # memset + init barrier).
    WpA = np.ascontiguousarray(np.concatenate([
        Wp.astype(np.float32).reshape(nt, P, C).transpose(1, 0, 2).reshape(P, nt * C),
        np.zeros((P, 1), np.float32)], axis=1))
    return c, np.ascontiguousarray(Inc), WpA, nt


# ----------------------------------------------------------------------------
# Device kernel
# ----------------------------------------------------------------------------

def _body_once(nc, tc, consts, prods_pool, psum_L, psum_out,
               d_outT, x_sb, inc_sb, wp_sb, F, C, b_shard, nt):
    # x_sb already holds x + c (host-shifted); min = 0.5 so no clamp.
    lx = consts.tile([F, b_shard], F32R)
    zb_x = x_sb[:, b_shard:b_shard + 1]      # shipped zero column
    zb_w = wp_sb[:, nt * C:nt * C + 1]       # shipped zero column
    if not NEWTON:
        nc.scalar.activation(lx, x_sb[:, :b_shard],
                             mybir.ActivationFunctionType.Ln, bias=zb_x)
    else:
        lx0 = consts.tile([F, b_shard], F32)
        nc.scalar.activation(lx0, x_sb, mybir.ActivationFunctionType.Ln)
        e_neg = consts.tile([F, b_shard], F32)
        nc.scalar.activation(e_neg, lx0, mybir.ActivationFunctionType.Exp,
                             scale=-1.0)
        corr = consts.tile([F, b_shard], F32)
        nc.vector.tensor_mul(out=corr, in0=x_sb, in1=e_neg)
        nc.vector.scalar_tensor_tensor(
            out=lx, in0=corr, scalar=1.0, in1=lx0,
            op0=mybir.AluOpType.subtract, op1=mybir.AluOpType.add)

    out_ps = psum_out.tile([C, b_shard], F32)
    G = -(-nt // EXP_FUSE)
    for g in range(G):
        w = min(EXP_FUSE, nt - g * EXP_FUSE)
        L_ps = psum_L.tile([P, EXP_FUSE * b_shard], F32, tag="L")
        for k in range(w):
            t = g * EXP_FUSE + k
            nc.tensor.matmul(
                L_ps[:, k * b_shard:(k + 1) * b_shard],
                inc_sb[:, t * P:(t + 1) * P],
                lx,
                start=True, stop=True)
        pg = prods_pool.tile([P, EXP_FUSE * b_shard], F32R, tag="pg")
        nc.scalar.activation(
            pg[:, :w * b_shard], L_ps[:, :w * b_shard],
            mybir.ActivationFunctionType.Exp, bias=zb_w)
        for k in range(w):
            t = g * EXP_FUSE + k
            nc.tensor.matmul(
                out_ps, wp_sb[:, t * C:(t + 1) * C],
                pg[:, k * b_shard:(k + 1) * b_shard],
                start=(t == 0), stop=(t == nt - 1))

    out_sb = consts.tile([C, b_shard], F32)
    nc.scalar.activation(out_sb, out_ps, mybir.ActivationFunctionType.Copy)
    nc.sync.dma_start(out=d_outT[:, :], in_=out_sb)


_nc_cache = {}


def _get_nc(F, C, b_shard, nt, c, repeat=1):
    key = (F, C, b_shard, nt, repeat)
    if key not in _nc_cache:
        _nc_cache[key] = _build_nc_with_c(F, C, b_shard, nt, c, repeat)
    return _nc_cache[key]


def _build_nc_with_c(F, C, b_shard, nt, c, repeat):
    nc = bacc.Bacc(None, target_bir_lowering=False)
    # inc carries [Inc | x'+c | zero-col] so input staging is one DMA fewer;
    # the x' slice is read back as plain f32 via bitcast (DMA is a byte copy).
    d_inc = nc.declare_dram_parameter(
        "inc", [F, nt * P + b_shard + 1], F32R, isOutput=False)
    d_wp = nc.declare_dram_parameter("wp", [P, nt * C + 1], F32R, isOutput=False)
    d_outT = nc.declare_dram_parameter("outT", [C, b_shard], F32, isOutput=True)

    with tile.TileContext(nc) as tc:
        with (
            tc.tile_pool(name="consts", bufs=1) as consts,
            tc.tile_pool(name="prods", bufs=3) as prods_pool,
            tc.tile_pool(name="psum_L", bufs=1, space="PSUM") as psum_L,
            tc.tile_pool(name="psum_out", bufs=1, space="PSUM") as psum_out,
        ):
            inc_sb = consts.tile([F, nt * P + b_shard + 1], F32R)
            nc.sync.dma_start(out=inc_sb, in_=d_inc[:, :])
            x_sb = inc_sb[:, nt * P:nt * P + b_shard + 1].bitcast(F32)
            wp_sb = consts.tile([P, nt * C + 1], F32R)
            nc.sync.dma_start(out=wp_sb, in_=d_wp[:, :])

            for _rep in range(repeat):
                _body_once(nc, tc, consts, prods_pool, psum_L, psum_out,
                           d_outT, x_sb, inc_sb, wp_sb, F, C, b_shard, nt)
    nc.finalize()
    _merge_act_table_loads(nc)
    _strip_overhead(nc)
    return nc


def _strip_overhead(nc):
    """Drop setup/tail instructions that don't affect this kernel's result:
    the unused const-AP memsets (only the f32 0.0 bias constant is read)
    and the end-block all-engine barrier cascade (drains / event semaphores
    that reference only barrier_* rendezvous semaphores).  The data-bearing
    completion waits (DMA / engine sems) are kept, so the output DMA is
    still guaranteed complete at NEFF end.  Validated under CoreSim's race
    detector."""
    blocks = list(nc.m.functions[0].blocks)
    for b in blocks:
        keep = []
        for inst in b.instructions:
            nm = type(inst).__name__
            si = inst.sync_info
            sems = []
            if si is not None:
                sems += [w.ant_name for w in si.on_wait]
                sems += [u.ant_name for u in si.on_update]
            only_barrier = bool(sems) and all(
                s.startswith("barrier_") for s in sems)
            if nm == "InstMemset" and "const-" in str(
                    getattr(inst.outs[0], "memref", "")):
                continue
            if nm in ("InstDrain", "InstEventSemaphore") and (
                    only_barrier or (nm == "InstDrain" and not sems)):
                continue
            # the remaining SP drain (wait ACT==all) is subsumed by the
            # out-DMA completion waits; InstISA is a Pool marker and no
            # Pool work remains after the memset removal
            if nm == "InstISA":
                continue
            if nm == "InstDrain":
                continue
            keep.append(inst)
        b.instructions[:] = keep
    # merge the (now purely sequential) blocks into one and drop the
    # inter-block branches; per-engine instruction order is preserved
    merged = []
    for b in blocks:
        for inst in b.instructions:
            if type(inst).__name__ == "InstUnconditionalBranch":
                continue
            merged.append(inst)
    blocks[0].instructions[:] = merged
    nc.m.functions[0].blocks[:] = [blocks[0]]


def _merge_act_table_loads(nc):
    """All activations here (Ln, Exp, Copy) live in one table set
    (natural_log_exp_and_others); keep a single load of that set instead
    of the per-function flip-flop bacc emits."""
    from concourse.hw_specs import get_activation_tables
    T = mybir.ActivationFunctionType
    tabs = get_activation_tables(nc.m.arch)
    combined = None
    for i, fns in enumerate(tabs.values()):
        if {T.Ln, T.Exp, T.Copy} <= fns:
            combined = i
            break
    if combined is None:
        return
    first = True
    for b in nc.m.functions[0].blocks:
        keep = []
        for inst in b.instructions:
            if isinstance(inst, mybir.InstLoadActFuncSet):
                if first:
                    inst.act_func_set_id = combined
                    first = False
                    keep.append(inst)
                # later loads dropped: set already resident
            else:
                keep.append(inst)
        b.instructions[:] = keep


def _make_in_maps(x, c, Inc, WpA, b_shard):
    F = x.shape[1]
    in_maps = []
    for i in range(N_CORES):
        shift = np.asarray(c, np.float64).reshape(1, -1)
        sh = np.ascontiguousarray(np.concatenate([
            Inc,
            (x[i * b_shard:(i + 1) * b_shard].astype(np.float64) + shift)
            .T.astype(np.float32),
            np.zeros((F, 1), np.float32)], axis=1))
        in_maps.append({"inc": sh, "wp": WpA})
    return in_maps


def kernel(x, bias, W1, W2, W3, idx1, idx2, idx3, _trace=False):
    x = np.asarray(x, np.float32)
    B, F = x.shape
    C = np.asarray(W1).shape[1]
    assert B % N_CORES == 0
    b_shard = B // N_CORES

    c, Inc, WpA, nt = _prepare(x, bias, W1, W2, W3, idx1, idx2, idx3)
    nc = _get_nc(F, C, b_shard, nt, c)
    in_maps = _make_in_maps(x, c, Inc, WpA, b_shard)
    res = run_bass_kernel_spmd(nc, in_maps, list(range(N_CORES)), trace=_trace)
    out = np.empty((B, C), np.float32)
    for i in range(N_CORES):
        out[i * b_shard:(i + 1) * b_shard] = res.results[i]["outT"].T
    if _trace:
        kernel.last_results = res
    return out
